# revision 16
# baseline (speedup 1.0000x reference)
"""Multi-head causal attention (B=4, T=2048, C=1024, H=16) on 8 trn2 cores.

Sharding: data-parallel over batch (4) x sequence-parallel over causal query
blocks (2), zig-zag balanced so all 8 cores run one identical program:
  core = 2*b + half;  half 0 gets query blocks [0,2,4,6,9,11,13,15],
  half 1 gets [1,3,5,7,8,10,12,14].  Slot s (0..7) processes J(s)=2s+2 key
  blocks; causal boundary handled by per-core input masks on the last two.
Each core writes a disjoint [1024, 1024] slice of the output; the host
scatters slices back and adds the (v/o-bias) correction  bo + bv @ Wo.T.
"""

import numpy as np
import ml_dtypes

import concourse.bass as bass
import concourse.mybir as mybir
import concourse.tile as tile
from concourse import bacc
from concourse.bass import ts
from concourse.bass_utils import run_bass_kernel_spmd

B, T, C, H, DK = 4, 2048, 1024, 16, 64
P = 128
NB = T // P          # 16 key blocks
SLOTS = 8            # query blocks per core
SCALE = 1.0 / np.sqrt(DK)
BF16 = mybir.dt.bfloat16
F32 = mybir.dt.float32
F32R = mybir.dt.float32r
EXP = mybir.ActivationFunctionType.Exp

QBLKS = [
    [0, 2, 4, 6, 9, 11, 13, 15],
    [1, 3, 5, 7, 8, 10, 12, 14],
]

_cache = {}


def _build():
    nc = bacc.Bacc("TRN2", target_bir_lowering=False, debug=False)

    xT = nc.dram_tensor("xT", [C, T], F32R, kind="ExternalInput").ap()
    xTq = nc.dram_tensor("xTq", [C, SLOTS * P], F32R, kind="ExternalInput").ap()
    wqT = nc.dram_tensor("wqT", [C, C], F32R, kind="ExternalInput").ap()
    wkT = nc.dram_tensor("wkT", [C, C], F32R, kind="ExternalInput").ap()
    wvT = nc.dram_tensor("wvT", [C, C], F32R, kind="ExternalInput").ap()
    woT = nc.dram_tensor("woT", [C, C], BF16, kind="ExternalInput").ap()
    bq = nc.dram_tensor("bq", [P, C // P], F32, kind="ExternalInput").ap()
    bk = nc.dram_tensor("bk", [P, C // P], F32, kind="ExternalInput").ap()
    masks = nc.dram_tensor("masks", [SLOTS, 2, P, P], BF16, kind="ExternalInput").ap()
    ident = nc.dram_tensor("ident", [P, P], BF16, kind="ExternalInput").ap()
    y = nc.dram_tensor("y", [SLOTS * P, C], F32, kind="ExternalOutput").ap()

    CB = C // P  # 8 column blocks of the channel dim

    with tile.TileContext(nc) as tc:
        with (
            tc.tile_pool(name="const", bufs=1) as cpool,
            tc.tile_pool(name="attn", bufs=1) as apool,
        ):
            # tiles allocated here; DMA emission deferred so the small
            # Q-phase loads head the gpsimd queue (fast PE start)
            masks_sb = cpool.tile([P, SLOTS, 2, P], BF16)
            ident_sb = cpool.tile([P, P], BF16)
            bq_sb = cpool.tile([P, CB], F32)
            bk_sb = cpool.tile([P, CB], F32)

            attn_out = apool.tile([P, SLOTS, C], BF16)

            with tc.tile_pool(name="qkv", bufs=1) as qkv:
                qT = qkv.tile([P, CB, SLOTS * P], BF16)
                kT = qkv.tile([P, CB, T], BF16)
                v = qkv.tile([P, NB, H * (DK + 1)], BF16)
                vg = v[:].rearrange("p t (h e) -> p t h e", e=DK + 1)
                nc.vector.memset(vg[:, :, :, DK : DK + 1], 1.0)

                # ---- phase 1: q/k/v projections (fp32r matmuls) ----
                # Each weight is DMA'd exactly once, split into two half
                # tiles (kb 0-3 / 4-7) so the next projection's first half
                # prefetches while the current one finishes (keeps PE warm).
                with (
                    tc.tile_pool(name="xt", bufs=1) as xt_pool,
                    tc.tile_pool(name="wres", bufs=1) as wres,
                    tc.tile_pool(name="xq", bufs=3) as xq_pool,
                    tc.tile_pool(name="pacc", bufs=1, space="PSUM") as pacc,
                ):
                    xT_sb = xt_pool.tile([P, CB, T], F32R)

                    def load_weight(src):
                        halves = []
                        for i, tag in enumerate(("wA", "wB")):
                            w_t = wres.tile([P, 4, C], F32R, tag=tag, name=tag)
                            nc.gpsimd.dma_start(
                                w_t[:],
                                src.rearrange("(ko p) n -> p ko n", p=P)[
                                    :, 4 * i : 4 * i + 4, :
                                ],
                            )
                            halves.append(w_t)
                        return lambda kb: halves[kb // 4][:, kb % 4, :]

                    # Q projection: qT[c_out, tq]
                    nc.gpsimd.dma_start(bq_sb[:], bq[:])
                    nc.gpsimd.dma_start(bk_sb[:], bk[:])
                    wq_at = load_weight(wqT)
                    for nch in range(2):
                        acc = [
                            pacc.tile([P, 512], F32, tag=f"acc{cb}", name=f"acc{cb}") for cb in range(CB)
                        ]
                        for kb in range(CB):
                            xq_ch = xq_pool.tile([P, 512], F32R, tag="xq")
                            nc.gpsimd.dma_start(
                                xq_ch[:],
                                xTq.rearrange("(ko p) t -> p ko t", p=P)[
                                    :, kb, ts(nch, 512)
                                ],
                            )
                            for cb in range(CB):
                                nc.tensor.matmul(
                                    acc[cb][:],
                                    wq_at(kb)[:, ts(cb, P)],
                                    xq_ch[:],
                                    start=(kb == 0),
                                    stop=(kb == CB - 1),
                                )
                        for cb in range(CB):
                            nc.vector.tensor_scalar_add(
                                qT[:, cb, ts(nch, 512)], acc[cb][:], bq_sb[:, cb : cb + 1]
                            )

                    # deferred big/const loads: emitted after Q's DMAs so the
                    # PE starts within ~8us; they overlap Q compute
                    nc.gpsimd.dma_start(
                        xT_sb[:], xT.rearrange("(ko p) t -> p ko t", p=P)
                    )
                    nc.gpsimd.dma_start(
                        masks_sb[:], masks[:].rearrange("s t p q -> p s t q")
                    )
                    nc.gpsimd.dma_start(ident_sb[:], ident[:])

                    # K projection: kT[c_out, t]
                    wk_at = load_weight(wkT)
                    for nch in range(4):
                        acc = [
                            pacc.tile([P, 512], F32, tag=f"acc{cb}", name=f"acc{cb}") for cb in range(CB)
                        ]
                        for kb in range(CB):
                            for cb in range(CB):
                                nc.tensor.matmul(
                                    acc[cb][:],
                                    wk_at(kb)[:, ts(cb, P)],
                                    xT_sb[:, kb, ts(nch, 512)],
                                    start=(kb == 0),
                                    stop=(kb == CB - 1),
                                )
                        for cb in range(CB):
                            nc.vector.tensor_scalar_add(
                                kT[:, cb, ts(nch, 512)], acc[cb][:], bk_sb[:, cb : cb + 1]
                            )

                    # V projection: v[t, d] natural layout, head-grouped with
                    # a ones column per head (free softmax denominator).
                    # Token blocks in groups of 4 so the 8 live accumulators
                    # fit PSUM.
                    wv_at = load_weight(wvT)
                    for tbg in range(NB // 4):
                        acc = [
                            pacc.tile([P, 512], F32, tag=f"acc{i}", name=f"vacc{i}")
                            for i in range(8)
                        ]
                        for kb in range(CB):
                            for ti in range(4):
                                tb = tbg * 4 + ti
                                for dch in range(2):
                                    nc.tensor.matmul(
                                        acc[ti * 2 + dch][:],
                                        xT_sb[:, kb, ts(tb, P)],
                                        wv_at(kb)[:, ts(dch, 512)],
                                        start=(kb == 0),
                                        stop=(kb == CB - 1),
                                    )
                        for ti in range(4):
                            tb = tbg * 4 + ti
                            for dch in range(2):
                                nc.vector.tensor_copy(
                                    vg[:, tb, dch * 8 : (dch + 1) * 8, 0:DK],
                                    acc[ti * 2 + dch][:].rearrange(
                                        "p (h e) -> p h e", e=DK
                                    ),
                                )

                # ---- phase 2: attention per head ----
                with (
                    tc.tile_pool(name="expS", bufs=3) as spool,
                    tc.tile_pool(name="small", bufs=8) as small,
                    tc.tile_pool(name="ps_s", bufs=3, space="PSUM") as ps_s,
                    tc.tile_pool(name="ps_o", bufs=2, space="PSUM") as ps_o,
                ):
                    for h in range(H):
                        hp = (h % 2) * DK
                        cbh = h // 2
                        expS = spool.tile([P, NB, SLOTS * P], BF16, tag="expS")
                        for jb in range(NB):
                            smin = jb // 2
                            q0 = smin * P
                            ncols = SLOTS * P - q0
                            # one 2-bank psum tile; matmuls fill 512-wide
                            # bank-aligned chunks, a single exp drains it
                            pss = ps_s.tile([P, SLOTS * P], F32, tag="ps_s")
                            cuts = sorted({q0, 512, SLOTS * P})
                            for a, b in zip(cuts, cuts[1:]):
                                if a < q0:
                                    continue
                                nc.tensor.matmul(
                                    pss[:, a:b],
                                    kT[hp : hp + DK, cbh, ts(jb, P)],
                                    qT[hp : hp + DK, cbh, a:b],
                                    start=True,
                                    stop=True,
                                )
                            nc.scalar.activation(
                                expS[:, jb, q0:], pss[:, q0:], EXP,
                                scale=float(SCALE),
                            )
                            # causal boundary: slot jb//2 sees jb as one of its
                            # last-two key blocks; mask multiplies after exp.
                            sm = jb // 2
                            nc.vector.tensor_mul(
                                expS[:, jb, ts(sm, P)],
                                expS[:, jb, ts(sm, P)],
                                masks_sb[:, sm, jb % 2, :],
                            )
                        for s in range(SLOTS):
                            J = 2 * s + 2
                            pso = ps_o.tile([P, DK + 1], F32, tag="ps_o")
                            for jb in range(J):
                                nc.tensor.matmul(
                                    pso[:],
                                    expS[:, jb, ts(s, P)],
                                    v[:, jb, h * (DK + 1) : (h + 1) * (DK + 1)],
                                    start=(jb == 0),
                                    stop=(jb == J - 1),
                                )
                            rec = small.tile([P, 1], F32, tag="rec")
                            nc.vector.reciprocal(rec[:], pso[:, DK : DK + 1])
                            nc.vector.tensor_scalar_mul(
                                attn_out[:, s, h * DK : (h + 1) * DK],
                                pso[:, 0:DK],
                                rec[:],
                            )

            # ---- phase 3: transpose + output projection ----
            with (
                tc.tile_pool(name="out", bufs=1) as opool,
                tc.tile_pool(name="ps_t", bufs=4, space="PSUM") as ps_t,
                tc.tile_pool(name="ps_y", bufs=2, space="PSUM") as ps_y,
            ):
                aT = opool.tile([P, CB, SLOTS * P], BF16)
                for cb in range(CB):
                    for s in range(SLOTS):
                        pst = ps_t.tile([P, P], BF16, tag="ps_t")
                        nc.tensor.transpose(
                            pst[:], attn_out[:, s, ts(cb, P)], ident_sb[:]
                        )
                        nc.vector.tensor_copy(aT[:, cb, ts(s, P)], pst[:])

                woT_sb = opool.tile([P, CB, C], BF16)
                nc.gpsimd.dma_start(
                    woT_sb[:], woT.rearrange("(ko p) n -> p ko n", p=P)
                )
                y_sb = opool.tile([P, SLOTS, C], F32)
                for tb in range(SLOTS):
                    for nch in range(2):
                        psy = ps_y.tile([P, 512], F32, tag="ps_y")
                        for cbk in range(CB):
                            nc.tensor.matmul(
                                psy[:],
                                aT[:, cbk, ts(tb, P)],
                                woT_sb[:, cbk, ts(nch, 512)],
                                start=(cbk == 0),
                                stop=(cbk == CB - 1),
                            )
                        nc.vector.tensor_copy(y_sb[:, tb, ts(nch, 512)], psy[:])
                nc.gpsimd.dma_start(
                    y.rearrange("(tb p) c -> p tb c", p=P), y_sb[:]
                )

    nc.compile()
    return nc


def _host_inputs(x, mask, Wq, bq_v, Wk, bk_v, Wv, bv_v, Wo, bo_v):
    """Per-core input maps + the host-side output bias correction."""
    f32 = np.float32
    bf16 = ml_dtypes.bfloat16
    wqT = np.ascontiguousarray(np.asarray(Wq, f32).T)
    wkT = np.ascontiguousarray(np.asarray(Wk, f32).T)
    wvT = np.ascontiguousarray(np.asarray(Wv, f32).T)
    woT = np.ascontiguousarray(np.asarray(Wo, f32).T).astype(bf16)
    bq_p = np.ascontiguousarray(np.asarray(bq_v, f32).reshape(C // P, P).T)
    bk_p = np.ascontiguousarray(np.asarray(bk_v, f32).reshape(C // P, P).T)
    identity = np.eye(P, dtype=f32).astype(bf16)
    # exact v/o bias fold: softmax rows sum to 1, so v+bv adds bv to attn out
    bo_eff = (np.asarray(bo_v, f32) + np.asarray(bv_v, f32) @ np.asarray(Wo, f32).T)

    # per-half causal boundary masks for the last two key blocks of each slot
    mask_half = []
    tri = np.tril(np.ones((P, P), f32)).T  # [j, i] = 1 where j <= i
    for half in range(2):
        m = np.zeros((SLOTS, 2, P, P), f32)
        for s in range(SLOTS):
            g = QBLKS[half][s]
            for idx, jb in enumerate((2 * s, 2 * s + 1)):
                if jb < g:
                    m[s, idx] = 1.0
                elif jb == g:
                    m[s, idx] = tri
        mask_half.append(m.astype(bf16))

    xn = np.asarray(x, f32)
    in_maps = []
    for core in range(8):
        b, half = divmod(core, 2)
        xT = np.ascontiguousarray(xn[b].T)
        qtok = np.concatenate([np.arange(g * P, (g + 1) * P) for g in QBLKS[half]])
        xTq = np.ascontiguousarray(xn[b][qtok].T)
        in_maps.append(
            {
                "xT": xT,
                "xTq": xTq,
                "wqT": wqT,
                "wkT": wkT,
                "wvT": wvT,
                "woT": woT,
                "bq": bq_p,
                "bk": bk_p,
                "masks": mask_half[half],
                "ident": identity,
            }
        )
    return in_maps, bo_eff


def _run(inputs, trace=False):
    if "nc" not in _cache:
        _cache["nc"] = _build()
    nc = _cache["nc"]
    in_maps, bo_eff = _host_inputs(
        inputs["x"], inputs["mask"],
        inputs["Wq"], inputs["bq"], inputs["Wk"], inputs["bk"],
        inputs["Wv"], inputs["bv"], inputs["Wo"], inputs["bo"],
    )
    res = run_bass_kernel_spmd(nc, in_maps, list(range(8)), trace=trace)
    out = np.empty((B, T, C), np.float32)
    for core in range(8):
        b, half = divmod(core, 2)
        yc = res.results[core]["y"]
        for s, g in enumerate(QBLKS[half]):
            out[b, g * P : (g + 1) * P] = yc[s * P : (s + 1) * P]
    out += bo_eff
    return out, res


def kernel(**inputs):
    out, _ = _run(inputs, trace=False)
    return out


# revision 17
# speedup vs baseline: 1.0677x; 1.0677x over previous
"""Multi-head causal attention (B=4, T=2048, C=1024, H=16) on 8 trn2 cores.

Sharding: data-parallel over batch (4) x sequence-parallel over causal query
blocks (2), zig-zag balanced so all 8 cores run one identical program:
  core = 2*b + half;  half 0 gets query blocks [0,2,4,6,9,11,13,15],
  half 1 gets [1,3,5,7,8,10,12,14].  Slot s (0..7) processes J(s)=2s+2 key
  blocks; causal boundary handled by per-core input masks on the last two.
Each core writes a disjoint [1024, 1024] slice of the output; the host
scatters slices back and adds the (v/o-bias) correction  bo + bv @ Wo.T.
"""

import numpy as np
import ml_dtypes

import concourse.bass as bass
import concourse.mybir as mybir
import concourse.tile as tile
from concourse import bacc
from concourse.bass import ts
from concourse.bass_utils import run_bass_kernel_spmd

B, T, C, H, DK = 4, 2048, 1024, 16, 64
P = 128
NB = T // P          # 16 key blocks
SLOTS = 8            # query blocks per core
SCALE = 1.0 / np.sqrt(DK)
BF16 = mybir.dt.bfloat16
F32 = mybir.dt.float32
F32R = mybir.dt.float32r
EXP = mybir.ActivationFunctionType.Exp

QBLKS = [
    [0, 2, 4, 6, 9, 11, 13, 15],
    [1, 3, 5, 7, 8, 10, 12, 14],
]

_cache = {}


def _build():
    nc = bacc.Bacc("TRN2", target_bir_lowering=False, debug=False)

    xT = nc.dram_tensor("xT", [C, T], BF16, kind="ExternalInput").ap()
    xTq = nc.dram_tensor("xTq", [C, SLOTS * P], BF16, kind="ExternalInput").ap()
    wqT = nc.dram_tensor("wqT", [C, C], BF16, kind="ExternalInput").ap()
    wkT = nc.dram_tensor("wkT", [C, C], BF16, kind="ExternalInput").ap()
    wvT = nc.dram_tensor("wvT", [C, C], BF16, kind="ExternalInput").ap()
    woT = nc.dram_tensor("woT", [C, C], BF16, kind="ExternalInput").ap()
    bq = nc.dram_tensor("bq", [P, C // P], F32, kind="ExternalInput").ap()
    bk = nc.dram_tensor("bk", [P, C // P], F32, kind="ExternalInput").ap()
    masks = nc.dram_tensor("masks", [SLOTS, 2, P, P], BF16, kind="ExternalInput").ap()
    ident = nc.dram_tensor("ident", [P, P], BF16, kind="ExternalInput").ap()
    y = nc.dram_tensor("y", [SLOTS * P, C], F32, kind="ExternalOutput").ap()

    CB = C // P  # 8 column blocks of the channel dim

    with tile.TileContext(nc) as tc:
        with (
            tc.tile_pool(name="const", bufs=1) as cpool,
            tc.tile_pool(name="attn", bufs=1) as apool,
        ):
            # tiles allocated here; DMA emission deferred so the small
            # Q-phase loads head the gpsimd queue (fast PE start)
            masks_sb = cpool.tile([P, SLOTS, 2, P], BF16)
            ident_sb = cpool.tile([P, P], BF16)
            bq_sb = cpool.tile([P, CB], F32)
            bk_sb = cpool.tile([P, CB], F32)

            attn_out = apool.tile([P, SLOTS, C], BF16)

            with tc.tile_pool(name="qkv", bufs=1) as qkv:
                qT = qkv.tile([P, CB, SLOTS * P], BF16)
                kT = qkv.tile([P, CB, T], BF16)
                v = qkv.tile([P, NB, H * (DK + 1)], BF16)
                vg = v[:].rearrange("p t (h e) -> p t h e", e=DK + 1)
                nc.vector.memset(vg[:, :, :, DK : DK + 1], 1.0)

                # ---- phase 1: q/k/v projections (fp32r matmuls) ----
                # Each weight is DMA'd exactly once, split into two half
                # tiles (kb 0-3 / 4-7) so the next projection's first half
                # prefetches while the current one finishes (keeps PE warm).
                with (
                    tc.tile_pool(name="xt", bufs=1) as xt_pool,
                    tc.tile_pool(name="wres", bufs=1) as wres,
                    tc.tile_pool(name="xq", bufs=3) as xq_pool,
                    tc.tile_pool(name="pacc", bufs=1, space="PSUM") as pacc,
                ):
                    xT_sb = xt_pool.tile([P, CB, T], BF16)

                    def load_weight(src):
                        halves = []
                        for i, tag in enumerate(("wA", "wB")):
                            w_t = wres.tile([P, 4, C], BF16, tag=tag, name=tag)
                            nc.gpsimd.dma_start(
                                w_t[:],
                                src.rearrange("(ko p) n -> p ko n", p=P)[
                                    :, 4 * i : 4 * i + 4, :
                                ],
                            )
                            halves.append(w_t)
                        return lambda kb: halves[kb // 4][:, kb % 4, :]

                    # Q projection: qT[c_out, tq]
                    nc.gpsimd.dma_start(bq_sb[:], bq[:])
                    nc.gpsimd.dma_start(bk_sb[:], bk[:])
                    wq_at = load_weight(wqT)
                    for nch in range(2):
                        acc = [
                            pacc.tile([P, 512], F32, tag=f"acc{cb}", name=f"acc{cb}") for cb in range(CB)
                        ]
                        for kb in range(CB):
                            xq_ch = xq_pool.tile([P, 512], BF16, tag="xq")
                            nc.gpsimd.dma_start(
                                xq_ch[:],
                                xTq.rearrange("(ko p) t -> p ko t", p=P)[
                                    :, kb, ts(nch, 512)
                                ],
                            )
                            for cb in range(CB):
                                nc.tensor.matmul(
                                    acc[cb][:],
                                    wq_at(kb)[:, ts(cb, P)],
                                    xq_ch[:],
                                    start=(kb == 0),
                                    stop=(kb == CB - 1),
                                )
                        for cb in range(CB):
                            nc.vector.tensor_scalar_add(
                                qT[:, cb, ts(nch, 512)], acc[cb][:], bq_sb[:, cb : cb + 1]
                            )

                    # deferred big/const loads: emitted after Q's DMAs so the
                    # PE starts within ~8us; they overlap Q compute
                    nc.gpsimd.dma_start(
                        xT_sb[:], xT.rearrange("(ko p) t -> p ko t", p=P)
                    )
                    nc.gpsimd.dma_start(
                        masks_sb[:], masks[:].rearrange("s t p q -> p s t q")
                    )
                    nc.gpsimd.dma_start(ident_sb[:], ident[:])

                    # K projection: kT[c_out, t]
                    wk_at = load_weight(wkT)
                    for nch in range(4):
                        acc = [
                            pacc.tile([P, 512], F32, tag=f"acc{cb}", name=f"acc{cb}") for cb in range(CB)
                        ]
                        for kb in range(CB):
                            for cb in range(CB):
                                nc.tensor.matmul(
                                    acc[cb][:],
                                    wk_at(kb)[:, ts(cb, P)],
                                    xT_sb[:, kb, ts(nch, 512)],
                                    start=(kb == 0),
                                    stop=(kb == CB - 1),
                                )
                        for cb in range(CB):
                            nc.vector.tensor_scalar_add(
                                kT[:, cb, ts(nch, 512)], acc[cb][:], bk_sb[:, cb : cb + 1]
                            )

                    # V projection: v[t, d] natural layout, head-grouped with
                    # a ones column per head (free softmax denominator).
                    # Token blocks in groups of 4 so the 8 live accumulators
                    # fit PSUM.
                    wv_at = load_weight(wvT)
                    for tbg in range(NB // 4):
                        acc = [
                            pacc.tile([P, 512], F32, tag=f"acc{i}", name=f"vacc{i}")
                            for i in range(8)
                        ]
                        for kb in range(CB):
                            for ti in range(4):
                                tb = tbg * 4 + ti
                                for dch in range(2):
                                    nc.tensor.matmul(
                                        acc[ti * 2 + dch][:],
                                        xT_sb[:, kb, ts(tb, P)],
                                        wv_at(kb)[:, ts(dch, 512)],
                                        start=(kb == 0),
                                        stop=(kb == CB - 1),
                                    )
                        for ti in range(4):
                            tb = tbg * 4 + ti
                            for dch in range(2):
                                nc.vector.tensor_copy(
                                    vg[:, tb, dch * 8 : (dch + 1) * 8, 0:DK],
                                    acc[ti * 2 + dch][:].rearrange(
                                        "p (h e) -> p h e", e=DK
                                    ),
                                )

                # ---- phase 2: attention per head ----
                with (
                    tc.tile_pool(name="expS", bufs=3) as spool,
                    tc.tile_pool(name="small", bufs=8) as small,
                    tc.tile_pool(name="ps_s", bufs=3, space="PSUM") as ps_s,
                    tc.tile_pool(name="ps_o", bufs=2, space="PSUM") as ps_o,
                ):
                    for h in range(H):
                        hp = (h % 2) * DK
                        cbh = h // 2
                        expS = spool.tile([P, NB, SLOTS * P], BF16, tag="expS")
                        for jb in range(NB):
                            smin = jb // 2
                            q0 = smin * P
                            ncols = SLOTS * P - q0
                            # one 2-bank psum tile; matmuls fill 512-wide
                            # bank-aligned chunks, a single exp drains it
                            pss = ps_s.tile([P, SLOTS * P], F32, tag="ps_s")
                            cuts = sorted({q0, 512, SLOTS * P})
                            for a, b in zip(cuts, cuts[1:]):
                                if a < q0:
                                    continue
                                nc.tensor.matmul(
                                    pss[:, a:b],
                                    kT[hp : hp + DK, cbh, ts(jb, P)],
                                    qT[hp : hp + DK, cbh, a:b],
                                    start=True,
                                    stop=True,
                                )
                            nc.scalar.activation(
                                expS[:, jb, q0:], pss[:, q0:], EXP,
                                scale=float(SCALE),
                            )
                            # causal boundary: slot jb//2 sees jb as one of its
                            # last-two key blocks; mask multiplies after exp.
                            sm = jb // 2
                            nc.vector.tensor_mul(
                                expS[:, jb, ts(sm, P)],
                                expS[:, jb, ts(sm, P)],
                                masks_sb[:, sm, jb % 2, :],
                            )
                        for s in range(SLOTS):
                            J = 2 * s + 2
                            pso = ps_o.tile([P, DK + 1], F32, tag="ps_o")
                            for jb in range(J):
                                nc.tensor.matmul(
                                    pso[:],
                                    expS[:, jb, ts(s, P)],
                                    v[:, jb, h * (DK + 1) : (h + 1) * (DK + 1)],
                                    start=(jb == 0),
                                    stop=(jb == J - 1),
                                )
                            rec = small.tile([P, 1], F32, tag="rec")
                            nc.vector.reciprocal(rec[:], pso[:, DK : DK + 1])
                            nc.vector.tensor_scalar_mul(
                                attn_out[:, s, h * DK : (h + 1) * DK],
                                pso[:, 0:DK],
                                rec[:],
                            )

            # ---- phase 3: transpose + output projection ----
            with (
                tc.tile_pool(name="out", bufs=1) as opool,
                tc.tile_pool(name="ps_t", bufs=4, space="PSUM") as ps_t,
                tc.tile_pool(name="ps_y", bufs=2, space="PSUM") as ps_y,
            ):
                aT = opool.tile([P, CB, SLOTS * P], BF16)
                for cb in range(CB):
                    for s in range(SLOTS):
                        pst = ps_t.tile([P, P], BF16, tag="ps_t")
                        nc.tensor.transpose(
                            pst[:], attn_out[:, s, ts(cb, P)], ident_sb[:]
                        )
                        nc.vector.tensor_copy(aT[:, cb, ts(s, P)], pst[:])

                woT_sb = opool.tile([P, CB, C], BF16)
                nc.gpsimd.dma_start(
                    woT_sb[:], woT.rearrange("(ko p) n -> p ko n", p=P)
                )
                y_sb = opool.tile([P, SLOTS, C], F32)
                for tb in range(SLOTS):
                    for nch in range(2):
                        psy = ps_y.tile([P, 512], F32, tag="ps_y")
                        for cbk in range(CB):
                            nc.tensor.matmul(
                                psy[:],
                                aT[:, cbk, ts(tb, P)],
                                woT_sb[:, cbk, ts(nch, 512)],
                                start=(cbk == 0),
                                stop=(cbk == CB - 1),
                            )
                        nc.vector.tensor_copy(y_sb[:, tb, ts(nch, 512)], psy[:])
                nc.gpsimd.dma_start(
                    y.rearrange("(tb p) c -> p tb c", p=P), y_sb[:]
                )

    nc.compile()
    return nc


def _host_inputs(x, mask, Wq, bq_v, Wk, bk_v, Wv, bv_v, Wo, bo_v):
    """Per-core input maps + the host-side output bias correction."""
    f32 = np.float32
    bf16 = ml_dtypes.bfloat16
    wqT = np.ascontiguousarray(np.asarray(Wq, f32).T).astype(bf16)
    wkT = np.ascontiguousarray(np.asarray(Wk, f32).T).astype(bf16)
    wvT = np.ascontiguousarray(np.asarray(Wv, f32).T).astype(bf16)
    woT = np.ascontiguousarray(np.asarray(Wo, f32).T).astype(bf16)
    bq_p = np.ascontiguousarray(np.asarray(bq_v, f32).reshape(C // P, P).T)
    bk_p = np.ascontiguousarray(np.asarray(bk_v, f32).reshape(C // P, P).T)
    identity = np.eye(P, dtype=f32).astype(bf16)
    # exact v/o bias fold: softmax rows sum to 1, so v+bv adds bv to attn out
    bo_eff = (np.asarray(bo_v, f32) + np.asarray(bv_v, f32) @ np.asarray(Wo, f32).T)

    # per-half causal boundary masks for the last two key blocks of each slot
    mask_half = []
    tri = np.tril(np.ones((P, P), f32)).T  # [j, i] = 1 where j <= i
    for half in range(2):
        m = np.zeros((SLOTS, 2, P, P), f32)
        for s in range(SLOTS):
            g = QBLKS[half][s]
            for idx, jb in enumerate((2 * s, 2 * s + 1)):
                if jb < g:
                    m[s, idx] = 1.0
                elif jb == g:
                    m[s, idx] = tri
        mask_half.append(m.astype(bf16))

    xn = np.asarray(x, f32)
    in_maps = []
    for core in range(8):
        b, half = divmod(core, 2)
        xT = np.ascontiguousarray(xn[b].T).astype(bf16)
        qtok = np.concatenate([np.arange(g * P, (g + 1) * P) for g in QBLKS[half]])
        xTq = np.ascontiguousarray(xn[b][qtok].T).astype(bf16)
        in_maps.append(
            {
                "xT": xT,
                "xTq": xTq,
                "wqT": wqT,
                "wkT": wkT,
                "wvT": wvT,
                "woT": woT,
                "bq": bq_p,
                "bk": bk_p,
                "masks": mask_half[half],
                "ident": identity,
            }
        )
    return in_maps, bo_eff


def _run(inputs, trace=False):
    if "nc" not in _cache:
        _cache["nc"] = _build()
    nc = _cache["nc"]
    in_maps, bo_eff = _host_inputs(
        inputs["x"], inputs["mask"],
        inputs["Wq"], inputs["bq"], inputs["Wk"], inputs["bk"],
        inputs["Wv"], inputs["bv"], inputs["Wo"], inputs["bo"],
    )
    res = run_bass_kernel_spmd(nc, in_maps, list(range(8)), trace=trace)
    out = np.empty((B, T, C), np.float32)
    for core in range(8):
        b, half = divmod(core, 2)
        yc = res.results[core]["y"]
        for s, g in enumerate(QBLKS[half]):
            out[b, g * P : (g + 1) * P] = yc[s * P : (s + 1) * P]
    out += bo_eff
    return out, res


def kernel(**inputs):
    out, _ = _run(inputs, trace=False)
    return out
